# revision 11
# baseline (speedup 1.0000x reference)
"""Trainium2 Bass kernel v3 for nn_BandProcessor.

Pair-level (256-token) processing, 64 token-tiles per core, batch on cores.
Key changes vs baseline (522us -> 476us HW):
  - residual adds fused into PSUM evacuation (DVE tensor_add reading PSUM),
    removing the separate ACT att-copies and standalone DVE adds
  - sqrt pair-batched on ACT; band matmuls trimmed (N=15 / N=1 edges)
  - out/in DMA per pair; ~2.5x fewer instructions than baseline
  - NOTE: APPLY_ENG must stay "dve" - real GPSIMD dispatch is us-scale
    (tried: 192 Pool applies doubled HW time to 1002us)
"""

import numpy as np
import ml_dtypes

import concourse.bacc as bacc
import concourse.mybir as mybir
from concourse.tile import TileContext
from concourse import bass_utils

B, T, D = 8, 8192, 256
H = 16
DECAY = 0.9
EPS = 1e-5
NT = T // 128           # 64 tiles
NP = NT // 2            # 32 pairs
SBP = 8                 # pairs per superblock (FFN batching) = 16 tiles
NSB = NP // SBP         # 4 superblocks

F32 = mybir.dt.float32
F32R = mybir.dt.float32r
BF16 = mybir.dt.bfloat16

AF = mybir.ActivationFunctionType
ALU = mybir.AluOpType

APPLY_ENG = "dve"   # "gpsimd" (Pool engine) or "dve"
MERGE_BANDS = True  # merged edge matmuls (HW-only: trips CoreSim pending-zero assert)
RES1_PE = True      # res1 via PE identity-matmul + ACT evac (off DVE)
GELU_FN = None         # override for CoreSim (no Gelu impl); None -> AF.Gelu


# ---------------------------------------------------------------- host prep

def _host_consts(inp):
    g1, b1_ = inp["n1_g"].astype(np.float64), inp["n1_b"].astype(np.float64)
    g2, b2_ = inp["n2_g"].astype(np.float64), inp["n2_b"].astype(np.float64)
    g3, b3_ = inp["n3_g"].astype(np.float64), inp["n3_b"].astype(np.float64)
    t_Wv, t_bv = inp["t_Wv"].astype(np.float64), inp["t_bv"].astype(np.float64)
    t_Wo, t_bo = inp["t_Wo"].astype(np.float64), inp["t_bo"].astype(np.float64)
    a_Wv, a_bv = inp["a_Wv"].astype(np.float64), inp["a_bv"].astype(np.float64)
    a_Wo, a_bo = inp["a_Wo"].astype(np.float64), inp["a_bo"].astype(np.float64)
    f_W1, f_b1 = inp["f_W1"].astype(np.float64), inp["f_b1"].astype(np.float64)
    f_W2, f_b2 = inp["f_W2"].astype(np.float64), inp["f_b2"].astype(np.float64)

    WtWo = t_Wv @ t_Wo
    WaWo = a_Wv @ a_Wo
    Wt_eff = (g1[:, None] * WtWo).astype(np.float32)
    bt_eff = (b1_ @ WtWo + t_bv @ t_Wo + t_bo).astype(np.float32)
    Wa_eff = (g2[:, None] * WaWo).astype(np.float32)
    ba_eff = (b2_ @ WaWo + a_bv @ a_Wo + a_bo).astype(np.float32)
    W1_eff = (g3[:, None] * f_W1).astype(np.float32)
    b1_eff = (b3_ @ f_W1 + f_b1).astype(np.float32)
    W2 = f_W2.astype(np.float32)
    b2 = f_b2.astype(np.float32)

    tw = DECAY ** np.arange(H, dtype=np.float64)
    tw = tw / tw.sum()
    w_lag = tw[::-1].copy()   # w_lag[d] weights h[t-d]

    band1c = np.zeros((128, 128), np.float64)
    for ti in range(128):
        for to in range(ti, min(128, ti + H)):
            band1c[ti, to] = w_lag[to - ti]
    # cross-tile part: prev-tile token p contributes to next-tile tout with
    # lag = tout + 128 - p in [1, 15] -> nonzero rows 113..127, cols 0..14
    band1p15 = np.zeros((128, 15), np.float64)
    for p in range(113, 128):
        for to in range(0, p - 112):
            band1p15[p, to] = w_lag[to + 128 - p]
    band2c = np.zeros((128, 128), np.float64)
    for ti in range(128):
        for to in range(max(0, ti - 1), min(128, ti + 2)):
            band2c[ti, to] = 1.0 / 3.0
    ep_col = np.zeros((128, 1), np.float64); ep_col[127, 0] = 1.0 / 3.0
    en_col = np.zeros((128, 1), np.float64); en_col[0, 0] = 1.0 / 3.0

    bf = lambda a: a.astype(ml_dtypes.bfloat16)

    c_t = np.cumsum(w_lag)[:H - 1]
    corr = ((c_t - 1.0)[:, None] * (b1_ @ WtWo)[None, :]).astype(np.float32)

    brow2 = np.stack([np.concatenate([b, b]) for b in (bt_eff, ba_eff, b2)])

    consts = {
        "wt": bf(np.stack([Wt_eff[0:128], Wt_eff[128:256]])),  # [2,128,256]
        "wa": bf(np.stack([Wa_eff[0:128], Wa_eff[128:256]])),
        "w1": bf(np.stack([W1_eff[0:128], W1_eff[128:256]])),  # [2,128,512]
        "w2": bf(np.stack([W2[k * 128:(k + 1) * 128] for k in range(4)])),
        "band1c": bf(band1c), "band1p15": bf(band1p15),
        "band2c": bf(band2c), "ep_col": bf(ep_col), "en_col": bf(en_col),
        "b1cp": bf(np.concatenate([band1c, band1p15], axis=1)),
        "b2cp": bf(np.concatenate([band2c, ep_col], axis=1)),
        "b2cn": bf(np.concatenate([en_col, band2c], axis=1)),
        "idf": np.eye(128, dtype=np.float32),
        "ones_r": np.ones((1, 128), np.float32),
        "brow2": brow2,                                        # [3,512]
        "b1col": b1_eff.reshape(4, 128).T.copy(),              # [128,4]
        "ident": np.eye(128, dtype=ml_dtypes.bfloat16),
    }
    need_corr = bool(np.abs(corr).max() > 0)
    return consts, corr, need_corr


# ---------------------------------------------------------------- bass build

def build_nc(repeat=1, need_corr=False):
    nc = bacc.Bacc("TRN2", target_bir_lowering=False, debug=False, num_devices=8)

    x_d = nc.dram_tensor("x", (T, D), F32R, kind="ExternalInput")
    out_d = nc.dram_tensor("out", (T, D), F32, kind="ExternalOutput")
    wt_d = nc.dram_tensor("wt", (2, 128, 256), BF16, kind="ExternalInput")
    wa_d = nc.dram_tensor("wa", (2, 128, 256), BF16, kind="ExternalInput")
    w1_d = nc.dram_tensor("w1", (2, 128, 512), BF16, kind="ExternalInput")
    w2_d = nc.dram_tensor("w2", (4, 128, 256), BF16, kind="ExternalInput")
    b1c_d = nc.dram_tensor("band1c", (128, 128), BF16, kind="ExternalInput")
    b1p_d = nc.dram_tensor("band1p15", (128, 15), BF16, kind="ExternalInput")
    b2c_d = nc.dram_tensor("band2c", (128, 128), BF16, kind="ExternalInput")
    epc_d = nc.dram_tensor("ep_col", (128, 1), BF16, kind="ExternalInput")
    enc_d = nc.dram_tensor("en_col", (128, 1), BF16, kind="ExternalInput")
    ones_d = nc.dram_tensor("ones_r", (1, 128), F32R, kind="ExternalInput")
    brow_d = nc.dram_tensor("brow2", (3, 512), F32R, kind="ExternalInput")
    b1col_d = nc.dram_tensor("b1col", (128, 4), F32, kind="ExternalInput")
    id_d = nc.dram_tensor("ident", (128, 128), BF16, kind="ExternalInput")
    b1cp_d = nc.dram_tensor("b1cp", (128, 143), BF16, kind="ExternalInput")
    b2cp_d = nc.dram_tensor("b2cp", (128, 129), BF16, kind="ExternalInput")
    b2cn_d = nc.dram_tensor("b2cn", (128, 129), BF16, kind="ExternalInput")
    idf_d = nc.dram_tensor("idf", (128, 128), F32R, kind="ExternalInput")
    corr_d = nc.dram_tensor("corr", (15, 256), F32, kind="ExternalInput") if need_corr else None

    with TileContext(nc) as tc:
        import contextlib
        ctx = contextlib.ExitStack()
        with ctx:
            consts = ctx.enter_context(tc.tile_pool(name="consts", bufs=1))
            xpool = ctx.enter_context(tc.tile_pool(name="xpool", bufs=8))
            xn1p = ctx.enter_context(tc.tile_pool(name="xn1p", bufs=4))
            a1p = ctx.enter_context(tc.tile_pool(name="a1p", bufs=3))
            x1p = ctx.enter_context(tc.tile_pool(name="x1p", bufs=10))
            xn2p = ctx.enter_context(tc.tile_pool(name="xn2p", bufs=4))
            a2p = ctx.enter_context(tc.tile_pool(name="a2p", bufs=3))
            x2p = ctx.enter_context(tc.tile_pool(name="x2p", bufs=14))
            xn3p = ctx.enter_context(tc.tile_pool(name="xn3p", bufs=3))
            xbp = ctx.enter_context(tc.tile_pool(name="xbp", bufs=2))
            gbp = ctx.enter_context(tc.tile_pool(name="gbp", bufs=2))
            outp = ctx.enter_context(tc.tile_pool(name="outp", bufs=2))
            smalls = ctx.enter_context(tc.tile_pool(name="smalls", bufs=6))
            psA = ctx.enter_context(tc.tile_pool(name="psA", bufs=1, space="PSUM"))
            psT = ctx.enter_context(tc.tile_pool(name="psT", bufs=1, space="PSUM"))
            psG = ctx.enter_context(tc.tile_pool(name="psG", bufs=2, space="PSUM"))
            psO = ctx.enter_context(tc.tile_pool(name="psO", bufs=1, space="PSUM"))

            # ---- constants
            wt_sb = consts.tile([128, 2, 256], BF16)
            wa_sb = consts.tile([128, 2, 256], BF16)
            w1_sb = consts.tile([128, 2, 512], BF16)
            w2_sb = consts.tile([128, 4, 256], BF16)
            for k in range(2):
                nc.sync.dma_start(out=wt_sb[:, k, :], in_=wt_d[k, :, :])
                nc.sync.dma_start(out=wa_sb[:, k, :], in_=wa_d[k, :, :])
                nc.sync.dma_start(out=w1_sb[:, k, :], in_=w1_d[k, :, :])
            for k in range(4):
                nc.sync.dma_start(out=w2_sb[:, k, :], in_=w2_d[k, :, :])
            b1c_sb = consts.tile([128, 128], BF16, tag="b1c")
            nc.sync.dma_start(out=b1c_sb, in_=b1c_d[:, :])
            b1p_sb = consts.tile([128, 15], BF16, tag="b1p")
            nc.sync.dma_start(out=b1p_sb, in_=b1p_d[:, :])
            b2c_sb = consts.tile([128, 128], BF16, tag="b2c")
            nc.sync.dma_start(out=b2c_sb, in_=b2c_d[:, :])
            epc_sb = consts.tile([128, 1], BF16, tag="epc")
            nc.sync.dma_start(out=epc_sb, in_=epc_d[:, :])
            enc_sb = consts.tile([128, 1], BF16, tag="enc")
            nc.sync.dma_start(out=enc_sb, in_=enc_d[:, :])
            ones_sb = consts.tile([1, 128], F32R, tag="ones")
            nc.sync.dma_start(out=ones_sb, in_=ones_d[:, :])
            brow_sb = consts.tile([1, 3, 512], F32R, tag="brow")
            nc.sync.dma_start(out=brow_sb, in_=brow_d[:, :])
            b1col_sb = consts.tile([128, 4], F32, tag="b1col")
            nc.sync.dma_start(out=b1col_sb, in_=b1col_d[:, :])
            id_sb = consts.tile([128, 128], BF16, tag="ident")
            nc.sync.dma_start(out=id_sb, in_=id_d[:, :])
            b1cp_sb = consts.tile([128, 143], BF16, tag="b1cp")
            nc.sync.dma_start(out=b1cp_sb, in_=b1cp_d[:, :])
            b2cp_sb = consts.tile([128, 129], BF16, tag="b2cp")
            nc.sync.dma_start(out=b2cp_sb, in_=b2cp_d[:, :])
            b2cn_sb = consts.tile([128, 129], BF16, tag="b2cn")
            nc.sync.dma_start(out=b2cn_sb, in_=b2cn_d[:, :])
            idf_sb = consts.tile([128, 128], F32R, tag="idf")
            nc.sync.dma_start(out=idf_sb, in_=idf_d[:, :])
            eps_sb = consts.tile([128, 1], F32, tag="eps")
            nc.vector.memset(eps_sb, EPS)
            corr_sb = None
            if need_corr:
                corr_sb = consts.tile([15, 256], F32, tag="corr")
                nc.sync.dma_start(out=corr_sb, in_=corr_d[:, :])

            st = {}
            from concourse.tile import add_dep_helper
            tab_state = {"last": None}

            def chain_tab(inst):
                if tab_state["last"] is not None:
                    add_dep_helper(inst.ins, tab_state["last"].ins,
                                   reason="act-table phase ordering")
                tab_state["last"] = inst

            GELU = GELU_FN if GELU_FN is not None else AF.Gelu

            # ---------------- stages (pair granularity) ----------------

            def dma_in(p):
                xp = xpool.tile([128, 2, 256], F32R, tag="x")
                lo = p * 256
                nc.sync.dma_start(
                    out=xp, in_=x_d[lo:lo + 256, :].rearrange("(a t) d -> t a d", a=2))
                st[("x", p)] = xp

            def stats(p, key, src):
                # src: [128, 2, 256] f32 SBUF tile
                s6 = smalls.tile([128, 2, 6], F32, tag=f"st{key}")
                mv = smalls.tile([128, 2, 2], F32, tag=f"mv{key}")
                for s in range(2):
                    nc.vector.bn_stats(s6[:, s, :], src[:, s, :])
                    nc.vector.bn_aggr(mv[:, s, :], s6[:, s, :])
                std = smalls.tile([128, 2], F32, tag=f"sd{key}")
                chain_tab(nc.scalar.activation(std, mv[:, :, 1], AF.Sqrt,
                                               bias=eps_sb[:, 0:1]))
                rstd = smalls.tile([128, 2], F32, tag=f"rs{key}")
                nc.vector.reciprocal(rstd, std)
                st[(f"mv{key}", p)] = mv
                st[(f"rs{key}", p)] = rstd

            apply_eng = nc.gpsimd if APPLY_ENG == "gpsimd" else nc.vector

            def apply_ln(p, key, src, pool):
                mv = st.pop((f"mv{key}", p))
                rstd = st.pop((f"rs{key}", p))
                xn = pool.tile([128, 2, 256], BF16, tag=f"xn{key}")
                for s in range(2):
                    apply_eng.tensor_scalar(out=xn[:, s, :], in0=src[:, s, :],
                                            scalar1=mv[:, s, 0:1],
                                            scalar2=rstd[:, s:s + 1],
                                            op0=ALU.subtract, op1=ALU.mult)
                st[(f"xn{key}", p)] = xn

            def band1(p):
                xn = st[("xn1", p)]
                xnm = st.get(("xn1", p - 1))
                agg = psA.tile([128, 2, 2, 128], F32, tag="agg1")
                first = True
                if MERGE_BANDS:
                    for h in range(2):
                        hs = slice(h * 128, (h + 1) * 128)
                        fl = agg[:, h, :, :].rearrange("p a b -> p (a b)")
                        nc.tensor.matmul(fl[:, 0:143], xn[:, 0, hs], b1cp_sb,
                                         start=first, stop=False)
                        first = False
                        nc.tensor.matmul(agg[:, h, 1, :], xn[:, 1, hs], b1c_sb,
                                         start=False,
                                         stop=(xnm is None and h == 1))
                else:
                    for h in range(2):
                        hs = slice(h * 128, (h + 1) * 128)
                        for s in range(2):
                            nc.tensor.matmul(agg[:, h, s, :], xn[:, s, hs], b1c_sb,
                                             start=first, stop=False)
                            first = False
                    for h in range(2):
                        hs = slice(h * 128, (h + 1) * 128)
                        nc.tensor.matmul(agg[:, h, 1, 0:15], xn[:, 0, hs], b1p_sb,
                                         start=False,
                                         stop=(xnm is None and h == 1))
                if xnm is not None:
                    for h in range(2):
                        hs = slice(h * 128, (h + 1) * 128)
                        nc.tensor.matmul(agg[:, h, 0, 0:15], xnm[:, 1, hs], b1p_sb,
                                         start=False, stop=(h == 1))
                st[("agg1", p)] = agg

            def evac1(p):
                agg = st.pop(("agg1", p))
                a1 = a1p.tile([128, 2, 2, 128], BF16, tag="a1")
                nc.scalar.activation(a1, agg, AF.Copy)
                st[("a1", p)] = a1

            def att1(p):
                a1 = st.pop(("a1", p))
                x1ps = psA.tile([128, 2, 256], F32, tag="x1ps")
                if RES1_PE:
                    xp = st.pop(("x", p))
                    nc.tensor.matmul(x1ps, idf_sb, xp, start=True, stop=False)
                    nc.tensor.matmul(x1ps, ones_sb, brow_sb[:, 0, :],
                                     start=False, stop=False)
                else:
                    nc.tensor.matmul(x1ps, ones_sb, brow_sb[:, 0, :],
                                     start=True, stop=False)
                for h in range(2):
                    for s in range(2):
                        nc.tensor.matmul(x1ps[:, s, :], a1[:, h, s, :],
                                         wt_sb[:, h, :], start=False,
                                         stop=(h == 1 and s == 1))
                st[("x1ps", p)] = x1ps

            def res1(p):
                x1ps = st.pop(("x1ps", p))
                x1 = x1p.tile([128, 2, 256], F32, tag="x1")
                if RES1_PE:
                    nc.scalar.activation(x1, x1ps, AF.Copy)
                else:
                    xp = st.pop(("x", p))
                    nc.vector.tensor_add(out=x1, in0=x1ps, in1=xp)
                if need_corr and p == 0:
                    nc.vector.tensor_add(out=x1[0:15, 0, :], in0=x1[0:15, 0, :],
                                         in1=corr_sb)
                st[("x1", p)] = x1

            def band2(p):
                # self-contained: uses xn2(p-1), xn2(p), xn2(p+1); all writes
                # into agg2(p) happen here, stop on the last one.
                xn = st[("xn2", p)]
                agg = psA.tile([128, 2, 2, 128], F32, tag="agg2")
                first = True
                if MERGE_BANDS:
                    for h in range(2):
                        hs = slice(h * 128, (h + 1) * 128)
                        fl = agg[:, h, :, :].rearrange("p a b -> p (a b)")
                        nc.tensor.matmul(fl[:, 0:129], xn[:, 0, hs], b2cp_sb,
                                         start=first, stop=False)
                        first = False
                        nc.tensor.matmul(fl[:, 127:256], xn[:, 1, hs], b2cn_sb,
                                         start=False, stop=False)
                else:
                    for h in range(2):
                        hs = slice(h * 128, (h + 1) * 128)
                        for s in range(2):
                            nc.tensor.matmul(agg[:, h, s, :], xn[:, s, hs], b2c_sb,
                                             start=first, stop=False)
                            first = False
                    for h in range(2):
                        hs = slice(h * 128, (h + 1) * 128)
                        # a[127] -> b[0]
                        nc.tensor.matmul(agg[:, h, 1, 0:1], xn[:, 0, hs], epc_sb,
                                         start=False, stop=False)
                        # b[0] -> a[127]
                        nc.tensor.matmul(agg[:, h, 0, 127:128], xn[:, 1, hs], enc_sb,
                                         start=False, stop=False)
                # left edge of a: prev pair's b[127] (or replicate pad x[0])
                xl = st[("xn2", p - 1)] if p > 0 else xn
                sl, cl = (1, epc_sb) if p > 0 else (0, enc_sb)
                for h in range(2):
                    hs = slice(h * 128, (h + 1) * 128)
                    nc.tensor.matmul(agg[:, h, 0, 0:1], xl[:, sl, hs], cl,
                                     start=False, stop=False)
                # right edge of b: next pair's a[0] (or replicate pad b[127])
                xr = st[("xn2", p + 1)] if p < NP - 1 else xn
                sr, cr = (0, enc_sb) if p < NP - 1 else (1, epc_sb)
                for h in range(2):
                    hs = slice(h * 128, (h + 1) * 128)
                    nc.tensor.matmul(agg[:, h, 1, 127:128], xr[:, sr, hs], cr,
                                     start=False, stop=(h == 1))
                st[("agg2", p)] = agg
                st.pop(("xn2", p - 1), None)

            def evac2(p):
                agg = st.pop(("agg2", p))
                a2 = a2p.tile([128, 2, 2, 128], BF16, tag="a2")
                nc.scalar.activation(a2, agg, AF.Copy)
                st[("a2", p)] = a2

            def att2(p):
                a2 = st.pop(("a2", p))
                x2ps = psA.tile([128, 2, 256], F32, tag="x2ps")
                nc.tensor.matmul(x2ps, ones_sb, brow_sb[:, 1, :],
                                 start=True, stop=False)
                for h in range(2):
                    for s in range(2):
                        nc.tensor.matmul(x2ps[:, s, :], a2[:, h, s, :],
                                         wa_sb[:, h, :], start=False,
                                         stop=(h == 1 and s == 1))
                st[("x2ps", p)] = x2ps

            def res2(p):
                x2ps = st.pop(("x2ps", p))
                x1 = st.pop(("x1", p))
                x2 = x2p.tile([128, 2, 256], F32, tag="x2")
                nc.vector.tensor_add(out=x2, in0=x2ps, in1=x1)
                st[("x2", p)] = x2

            def trans(p):
                xn = st.pop(("xn3", p))
                x3ps = psT.tile([128, 2, 2, 128], BF16, tag="x3ps")
                for h in range(2):
                    hs = slice(h * 128, (h + 1) * 128)
                    for s in range(2):
                        nc.tensor.transpose(x3ps[:, h, s, :], xn[:, s, hs], id_sb)
                sb = p // SBP
                u = p % SBP
                if u == 0:
                    xbuf_t = xbp.tile([128, 2, 16, 128], BF16, tag="xbuf")
                    st[("xbuf", sb)] = xbuf_t
                xb = st[("xbuf", sb)]
                nc.scalar.activation(
                    xb[:, :, 2 * u:2 * u + 2, :], x3ps, AF.Copy)

            def ffn(sb):
                xb = st.pop(("xbuf", sb))
                gb = gbp.tile([128, 4, 2048], BF16, tag="gbuf")
                st[("gbuf", sb)] = gb
                for q in range(4):
                    qs = slice(q * 512, (q + 1) * 512)
                    qs4 = slice(q * 4, (q + 1) * 4)
                    for m in range(4):
                        gps = psG.tile([128, 512], F32, tag="gps")
                        ms = slice(m * 128, (m + 1) * 128)
                        nc.tensor.matmul(gps, w1_sb[:, 0, ms],
                                         xb[:, 0, qs4, :], start=True, stop=False)
                        nc.tensor.matmul(gps, w1_sb[:, 1, ms],
                                         xb[:, 1, qs4, :], start=False, stop=True)
                        chain_tab(nc.scalar.activation(gb[:, m, qs], gps, GELU,
                                                       bias=b1col_sb[:, m:m + 1]))

            def w2out(p):
                sb = p // SBP
                gb = st[("gbuf", sb)]
                ops = psO.tile([128, 2, 256], F32, tag="ops")
                nc.tensor.matmul(ops, ones_sb, brow_sb[:, 2, :],
                                 start=True, stop=False)
                for s in range(2):
                    t = (p % SBP) * 2 + s
                    cs = slice(t * 128, (t + 1) * 128)
                    for k in range(4):
                        nc.tensor.matmul(ops[:, s, :], gb[:, k, cs],
                                         w2_sb[:, k, :], start=False,
                                         stop=(s == 1 and k == 3))
                x2 = st.pop(("x2", p))
                ot = outp.tile([128, 2, 256], F32, tag="out")
                nc.vector.tensor_add(out=ot, in0=ops, in1=x2)
                lo = p * 256
                nc.sync.dma_start(
                    out=out_d[lo:lo + 256, :].rearrange("(a t) d -> t a d", a=2),
                    in_=ot)

            # ---------------- emission: software pipeline ----------------

            def body():
                st.clear()
                stages = [
                    (lambda p: dma_in(p), 0),
                    (lambda p: stats(p, "1", st[("x", p)].bitcast(F32)), 1),
                    (lambda p: apply_ln(p, "1", st[("x", p)].bitcast(F32), xn1p), 2),
                    (lambda p: band1(p), 3),
                    (lambda p: evac1(p), 4),
                    (lambda p: att1(p), 5),
                    (lambda p: res1(p), 6),
                    (lambda p: stats(p, "2", st[("x1", p)]), 7),
                    (lambda p: apply_ln(p, "2", st[("x1", p)], xn2p), 9),
                    (lambda p: band2(p), 11),
                    (lambda p: evac2(p), 12),
                    (lambda p: att2(p), 13),
                    (lambda p: res2(p), 14),
                    (lambda p: stats(p, "3", st[("x2", p)]), 15),
                    (lambda p: apply_ln(p, "3", st[("x2", p)], xn3p), 17),
                    (lambda p: trans(p), 18),
                ]
                # cleanup of xn1 after band1(p+1) consumed it
                def clean(p):
                    st.pop(("xn1", p - 1), None)
                stages.append((clean, 4))

                import collections as _c
                w2q = _c.deque()
                max_step = NP + 40
                for s_ in range(max_step):
                    for fn, d_ in reversed(stages):
                        i = s_ - d_
                        if 0 <= i < NP:
                            fn(i)
                    # FFN trigger after the step's LN sqrts are queued, so the
                    # act-table chain doesn't stall them behind the gelu burst
                    pff = s_ - 19
                    if pff >= 0 and pff % SBP == SBP - 1 and pff < NP:
                        ffn(pff // SBP)
                        w2q.extend(range(pff - SBP + 1, pff + 1))
                    if w2q:
                        w2out(w2q.popleft())
                while w2q:
                    w2out(w2q.popleft())

            if repeat > 1:
                with tc.For_i(0, repeat, 1):
                    body()
            else:
                body()

    nc.compile()
    return nc


# ---------------------------------------------------------------- entry

def _run(inputs, repeat=1, n_calls=1):
    import time
    consts, corr, need_corr = _host_consts(inputs)
    nc = build_nc(repeat=repeat, need_corr=need_corr)
    x = np.asarray(inputs["x"], np.float32)
    in_maps = []
    for b in range(B):
        m = {"x": np.ascontiguousarray(x[b])}
        for k, v in consts.items():
            m[k] = v
        if need_corr:
            m["corr"] = corr
        in_maps.append(m)
    times = []
    res = None
    for _ in range(n_calls):
        t0 = time.time()
        res = bass_utils.run_bass_kernel_spmd(nc, in_maps, core_ids=list(range(B)))
        times.append(time.time() - t0)
    out = np.stack([res.results[b]["out"] for b in range(B)]).astype(np.float32)
    return out, times


def kernel(**inputs) -> np.ndarray:
    try:
        out, _ = _run(inputs, repeat=1, n_calls=1)
    except Exception:
        out, _ = _run(inputs, repeat=1, n_calls=1)
    return out
